# revision 34
# baseline (speedup 1.0000x reference)
"""Trainium2 Bass kernel: multi-head attention (B=4, T=2048, D=2048, H=16).

Sharding: 8 cores = 4 batches x 2 head-groups (tensor-parallel heads, data-
parallel batch). Each core handles one batch and 8 heads (f-slice of 1024
columns of the QKV projections / rows of the out-projection). Host sums the
two partial out-projection results per batch and adds the output bias.

Structure (v9):
  v-pass (bf16, dc-outer start so matmuls pace with DMA arrival), then
  q/k passes in fp8-e4m3 DoubleRow (256-contraction pair matmuls, weights
  host-prescaled by 64, descaled in the bias ACT; adds ~1.6e-2 rel err,
  inside the 2e-2 budget).  Attention units (head, q-half) are emitted
  half-major and software-pipelined (scores(u+1) before pv(u)) so the
  ScalarE exp stream never blocks the PE queue: S^T strips -> exp -> P^T;
  PV with ones-augmented V gives rowsums in col 129; normalize on DVE and
  DMA-xbar-transpose straight into yT (v bias folded into the host
  combine: softmax rows sum to 1, so out += Wo @ bv, a constant).  The
  out-projection for t-blocks 0/1 interleaves into the last units' PE
  slack (B is ScalarE-bound); t-blocks 2/3 run after.  DRAM layouts match
  SBUF tiles for large DMA packets; yT/pt pools reuse the w/x SBUF region.
"""

import sys

if "/opt/trn_rl_repo" not in sys.path:
    sys.path.insert(0, "/opt/trn_rl_repo")

import numpy as np
import ml_dtypes

D = 2048          # d_model
T = 2048          # sequence length
B = 4             # batch
H = 16            # total heads
DH = 128          # head dim
GROUPS = 2        # head groups (tensor-parallel factor per batch)
HG = H // GROUPS  # heads per core = 8
F = HG * DH       # per-core projection width = 1024
P = 128
DC = D // P       # 16 contraction chunks
DCP = DC // 2     # 8 fp8 DoubleRow pair-chunks
TC = T // P       # 16 t chunks
NCORES = 8
SCALE = float(1.0 / np.sqrt(DH))
W8SCALE = 64.0    # host prescale for fp8 weights (avoids e4m3 subnormals)


_PROGRAM = None


def _build_program():
    import concourse.bass as bass
    import concourse.tile as tile
    from concourse import bacc, mybir
    from concourse.bass import ts, ds
    from concourse.masks import make_identity

    bf16 = mybir.dt.bfloat16
    f32 = mybir.dt.float32
    f8 = mybir.dt.float8e4
    DR = mybir.MatmulPerfMode.DoubleRow

    nc = bacc.Bacc("TRN2", target_bir_lowering=False, debug=False,
                   num_devices=NCORES)

    # DRAM layouts match the SBUF tile layouts exactly (contiguous
    # per-partition runs -> large DMA packets).
    xT_d = nc.dram_tensor("xT", [4, P, DC, 512], bf16, kind="ExternalInput")
    x8_d = nc.dram_tensor("x8", [4, P, DCP, 2, 512], f8, kind="ExternalInput")
    wq_d = nc.dram_tensor("wq", [P, DCP, 2, F], f8, kind="ExternalInput")
    wk_d = nc.dram_tensor("wk", [P, DCP, 2, F], f8, kind="ExternalInput")
    wv_d = nc.dram_tensor("wv", [P, DC, F], bf16, kind="ExternalInput")
    wo_d = nc.dram_tensor("wo", [DC, P, HG, P], bf16, kind="ExternalInput")
    bq_d = nc.dram_tensor("bq", [P, HG], f32, kind="ExternalInput")
    bk_d = nc.dram_tensor("bk", [P, HG], f32, kind="ExternalInput")
    out_d = nc.dram_tensor("out", [DC, P, T], f32, kind="ExternalOutput")

    Exp = mybir.ActivationFunctionType.Exp
    Identity = mybir.ActivationFunctionType.Identity

    with tile.TileContext(nc) as tc:
        from contextlib import ExitStack
        with ExitStack() as ctx:
            # ---- persistent pools (allocated first, live whole kernel) ----
            const = ctx.enter_context(tc.tile_pool(name="const", bufs=1))
            qkt = ctx.enter_context(tc.tile_pool(name="qkt", bufs=1))
            vpool = ctx.enter_context(tc.tile_pool(name="vpool", bufs=1))

            zero_b = const.tile([P, 1], f32, tag="zerob")
            nc.vector.memset(zero_b[:], 0.0)
            bq_sb = const.tile([P, HG], f32, tag="bq")
            bk_sb = const.tile([P, HG], f32, tag="bk")

            qT = [qkt.tile([P, T], bf16, tag=f"qT{h}", name=f"qT{h}")
                  for h in range(HG)]
            kT = [qkt.tile([P, T], bf16, tag=f"kT{h}", name=f"kT{h}")
                  for h in range(HG)]
            v_sb = vpool.tile([P, TC, HG, DH + 1], bf16, tag="v")

            # force early allocation of persistent pools (first-use order)
            nc.vector.memset(qT[0][:, 0:1], 0.0)
            nc.vector.memset(v_sb[:, :, :, DH:DH + 1], 1.0)

            # ---------------- Phase A: projections ----------------
            with tc.tile_pool(name="wall", bufs=1) as wall, \
                 tc.tile_pool(name="x8p", bufs=2) as x8pool, \
                 tc.tile_pool(name="ps_a", bufs=8, space="PSUM") as ps_a:
                w_v = wall.tile([P, DC, F], bf16, tag="wv", name="w_v")
                w_q8 = wall.tile([P, DCP, 2, F], f8, tag="wq8", name="w_q8")

                # ---- v-pass (bf16, x streamed) ----
                # x8p allocated before vx so their regions don't overlap
                # (overlap would stall the x8 prefetch until v-pass ends).
                # wk8 is loaded later, into the region vx frees.
                x8_first = x8pool.tile([P, DCP, 2, 512], f8, tag="x8",
                                       name="x8_q0")
                with tc.tile_pool(name="vx", bufs=2) as vxpool:
                    xb0 = vxpool.tile([P, DC, 512], bf16, tag="xblk",
                                      name="xv0")
                    # startup: interleave wv and x chunks per-dc so the
                    # dc-outer matmuls pace with DMA arrival
                    for dc in range(DC):
                        nc.sync.dma_start(w_v[:, ds(dc, 1)],
                                          wv_d[:, ds(dc, 1)])
                        nc.sync.dma_start(xb0[:, ds(dc, 1)],
                                          xT_d[0, :, ds(dc, 1)])
                        if dc == 9:
                            xb1 = vxpool.tile([P, DC, 512], bf16, tag="xblk",
                                              name="xv1")
                            nc.sync.dma_start(xb1[:], xT_d[1])
                    # fp8 q weights + first x8 block: prefetch during v
                    nc.sync.dma_start(w_q8[:], wq_d[:])
                    nc.sync.dma_start(x8_first[:], x8_d[0])
                    nc.sync.dma_start(bq_sb[:], bq_d[:])
                    nc.sync.dma_start(bk_sb[:], bk_d[:])

                    for tcb in range(4):
                        if tcb == 0:
                            xblk = xb0
                        elif tcb == 1:
                            xblk = xb1
                        else:
                            xblk = vxpool.tile([P, DC, 512], bf16, tag="xblk",
                                               name=f"xv{tcb}")
                            nc.sync.dma_start(xblk[:], xT_d[tcb])
                        if tcb == 0:
                            # dc-outer so compute paces with DMA arrival
                            pls = [ps_a.tile([P, 512], f32, tag="ps512",
                                             name=f"v0l{t}") for t in range(4)]
                            prs = [ps_a.tile([P, 512], f32, tag="ps512",
                                             name=f"v0r{t}") for t in range(4)]
                            for dc in range(DC):
                                for tsub in range(4):
                                    lhs = xblk[:, dc, ds(tsub * P, P)]
                                    nc.tensor.matmul(
                                        pls[tsub][:], lhs, w_v[:, dc, 0:512],
                                        start=(dc == 0), stop=(dc == DC - 1))
                                    nc.tensor.matmul(
                                        prs[tsub][:], lhs, w_v[:, dc, 512:1024],
                                        start=(dc == 0), stop=(dc == DC - 1))
                            for tsub in range(4):
                                nc.vector.tensor_copy(
                                    v_sb[:, tsub, 0:4, 0:DH],
                                    pls[tsub][:].rearrange(
                                        "p (h d) -> p h d", d=DH))
                                nc.vector.tensor_copy(
                                    v_sb[:, tsub, 4:8, 0:DH],
                                    prs[tsub][:].rearrange(
                                        "p (h d) -> p h d", d=DH))
                        else:
                            for tsub in range(4):
                                tc_ = tcb * 4 + tsub
                                psl = ps_a.tile([P, 512], f32, tag="ps512",
                                                name=f"psl{tc_}")
                                psr = ps_a.tile([P, 512], f32, tag="ps512",
                                                name=f"psr{tc_}")
                                for dc in range(DC):
                                    lhs = xblk[:, dc, ds(tsub * P, P)]
                                    nc.tensor.matmul(
                                        psl[:], lhs, w_v[:, dc, 0:512],
                                        start=(dc == 0), stop=(dc == DC - 1))
                                    nc.tensor.matmul(
                                        psr[:], lhs, w_v[:, dc, 512:1024],
                                        start=(dc == 0), stop=(dc == DC - 1))
                                nc.vector.tensor_copy(
                                    v_sb[:, tc_, 0:4, 0:DH],
                                    psl[:].rearrange("p (h d) -> p h d", d=DH))
                                nc.vector.tensor_copy(
                                    v_sb[:, tc_, 4:8, 0:DH],
                                    psr[:].rearrange("p (h d) -> p h d", d=DH))

                # ---- q/k passes (fp8 DoubleRow, 256-contraction) ----
                # wk8 loads into the region vx freed, during the q-pass
                with tc.tile_pool(name="wk8p", bufs=1) as wk8pool:
                    w_k8 = wk8pool.tile([P, DCP, 2, F], f8, tag="wk8",
                                        name="w_k8")
                    nc.sync.dma_start(w_k8[:], wk_d[:])
                    for kind in ("q", "k"):
                        w8 = w_q8 if kind == "q" else w_k8
                        bias_sb = bq_sb if kind == "q" else bk_sb
                        dst = qT if kind == "q" else kT
                        for tcb in range(4):
                            if kind == "q" and tcb == 0:
                                x8 = x8_first
                            else:
                                x8 = x8pool.tile([P, DCP, 2, 512], f8,
                                                 tag="x8",
                                                 name=f"x8_{kind}{tcb}")
                                nc.sync.dma_start(x8[:], x8_d[tcb])
                            for h in range(HG):
                                ps = ps_a.tile([P, 512], f32, tag="ps512",
                                               name=f"ps_{kind}{tcb}{h}")
                                for j in range(DCP):
                                    nc.tensor.matmul(
                                        ps[:],
                                        w8[:, j, :, ds(h * DH, DH)],
                                        x8[:, j],
                                        start=(j == 0), stop=(j == DCP - 1),
                                        perf_mode=DR)
                                nc.scalar.activation(
                                    dst[h][:, ds(tcb * 512, 512)], ps[:],
                                    Identity, bias=bias_sb[:, ds(h, 1)],
                                    scale=1.0 / W8SCALE)

            # ---------------- Phase B: attention ----------------
            # yT + pt reuse the SBUF region freed by the A pools.
            ytp = ctx.enter_context(tc.tile_pool(name="ytp", bufs=1))
            yT = ytp.tile([P, HG, T], bf16, tag="yT")
            wop = ctx.enter_context(tc.tile_pool(name="wop", bufs=3))
            osb = ctx.enter_context(tc.tile_pool(name="osb", bufs=3))
            wocp = ctx.enter_context(tc.tile_pool(name="wocp", bufs=1))
            with tc.tile_pool(name="ptpool", bufs=2) as ptpool, \
                 tc.tile_pool(name="ystage", bufs=4) as ystage, \
                 tc.tile_pool(name="rspool", bufs=4) as rspool, \
                 tc.tile_pool(name="ps_st", bufs=2, space="PSUM") as ps_st, \
                 tc.tile_pool(name="ps_pv", bufs=2, space="PSUM") as ps_pv, \
                 tc.tile_pool(name="ps_o", bufs=2, space="PSUM") as ps_o:

                def emit_scores(h, half):
                    # S^T[k=128, q=1024] strips; exp -> P^T
                    q0 = half * (T // 2)
                    pt = ptpool.tile([P, TC, T // 2], bf16, tag="pt",
                                     name=f"pt{h}_{half}")
                    for kc in range(TC):
                        st = ps_st.tile([P, T // 2], f32, tag="st",
                                        name=f"st{h}{half}{kc}")
                        for qc in range(2):
                            nc.tensor.matmul(
                                st[:, ds(qc * 512, 512)],
                                kT[h][:, ds(kc * P, P)],
                                qT[h][:, ds(q0 + qc * 512, 512)],
                                start=True, stop=True)
                        nc.scalar.activation(pt[:, kc], st[:], Exp,
                                             bias=zero_b[:, :],
                                             scale=SCALE)
                    return pt

                def emit_pv(h, half, pt):
                    # PV: out[q=128, dh | rowsum]; normalize; DMA-xbar
                    # transpose straight into yT (bv folded into the host
                    # combine since softmax rows sum to 1)
                    q0 = half * (T // 2)
                    for qs in range(8):
                        pv = ps_pv.tile([P, DH + 1], f32, tag="pv",
                                        name=f"pv{h}{half}{qs}")
                        for kc in range(TC):
                            nc.tensor.matmul(
                                pv[:],
                                pt[:, kc, ds(qs * P, P)],
                                v_sb[:, kc, h],
                                start=(kc == 0), stop=(kc == TC - 1))
                        rs = rspool.tile([P, 1], f32, tag="rs",
                                         name=f"rs{h}{half}{qs}")
                        nc.vector.reciprocal(rs[:], pv[:, DH:DH + 1])
                        yst = ystage.tile([P, P], bf16, tag="yst",
                                          name=f"yst{h}{half}{qs}")
                        nc.vector.tensor_scalar_mul(yst[:], pv[:, 0:DH],
                                                    rs[:])
                        nc.sync.dma_start_transpose(
                            yT[:, h, ds(q0 + qs * P, P)], yst[:])

                def emit_outproj(dch, tcbs, wo_t):
                    # partial out-projection for t-blocks `tcbs`
                    for tcb in tcbs:
                        pso = ps_o.tile([P, 512], f32, tag="pso",
                                        name=f"pso{dch}_{tcb}")
                        for fc in range(HG):
                            nc.tensor.matmul(
                                pso[:],
                                wo_t[:, fc],
                                yT[:, fc, ds(tcb * 512, 512)],
                                start=(fc == 0), stop=(fc == HG - 1))
                        ot = osb.tile([P, 512], f32, tag="ot",
                                      name=f"ot{dch}_{tcb}")
                        nc.vector.tensor_copy(ot[:], pso[:])
                        nc.sync.dma_start(
                            out_d[dch, :, ds(tcb * 512, 512)], ot[:])

                # software pipeline, half-major: after the 8 half0 units,
                # yT[:, :, 0:1024] is complete, so the out-projection for
                # t-blocks 0-1 interleaves into the remaining units' PE
                # slack (B is ScalarE-bound).
                units = [(h, half) for half in range(2) for h in range(HG)]
                # C(t-blocks 0,1) interleave into units 9+ once half0 of
                # yT is complete; ~11 dch fit the ScalarE-bound B slack
                # (PE-B 229.6us + 38us of C ~= exp 268us), tapered so the
                # ScalarE lead buffer absorbs the PE overshoot.
                cplan = {i: 2 for i in range(9, 16)}
                prev = None
                next_dch = 0
                woc0 = None
                for i, (h, half) in enumerate(units):
                    pt = emit_scores(h, half)
                    if prev is not None:
                        emit_pv(prev[0], prev[1], prev[2])
                    prev = (h, half, pt)
                    for _ in range(cplan.get(i, 0)):
                        wo_t = wop.tile([P, HG, P], bf16, tag="wo",
                                        name=f"wo{next_dch}")
                        nc.sync.dma_start(wo_t[:], wo_d[next_dch])
                        emit_outproj(next_dch, (0, 1), wo_t)
                        next_dch += 1
                    if i == 14:
                        # dedicated prefetch of the C-tail's first wo so
                        # phase C isn't gated on a rotation-blocked DMA
                        woc0 = wocp.tile([P, HG, P], bf16, tag="woc0",
                                         name="woc0")
                        nc.sync.dma_start(woc0[:], wo_d[0])
                emit_pv(prev[0], prev[1], prev[2])
                for dch in range(next_dch, DC):
                    wo_t = wop.tile([P, HG, P], bf16, tag="wo",
                                    name=f"wo{dch}")
                    nc.sync.dma_start(wo_t[:], wo_d[dch])
                    emit_outproj(dch, (0, 1), wo_t)

            # ------------- Phase C: out-projection t-blocks 2,3 -------------
            # dedicated deep pools: the pt pool's 64KB is free during C,
            # so weight prefetch and out staging never rotation-stall here
            with tc.tile_pool(name="ps_c", bufs=8, space="PSUM") as ps_c, \
                 tc.tile_pool(name="wop_c", bufs=6) as wop_c, \
                 tc.tile_pool(name="osb_c", bufs=8) as osb_c:
                for dch in range(DC):
                    if dch == 0:
                        wo_t = woc0
                    else:
                        wo_t = wop_c.tile([P, HG, P], bf16, tag="wo",
                                          name=f"wo_c{dch}")
                        nc.sync.dma_start(wo_t[:], wo_d[dch])
                    pso = [ps_c.tile([P, 512], f32, tag="psoc",
                                     name=f"psoc{dch}_{i}")
                           for i in range(2)]
                    for fc in range(HG):
                        for i, tcb in enumerate((2, 3)):
                            nc.tensor.matmul(
                                pso[i][:],
                                wo_t[:, fc],
                                yT[:, fc, ds(tcb * 512, 512)],
                                start=(fc == 0), stop=(fc == HG - 1))
                    for i, tcb in enumerate((2, 3)):
                        ot = osb_c.tile([P, 512], f32, tag="ot",
                                        name=f"otc{dch}_{tcb}")
                        nc.vector.tensor_copy(ot[:], pso[i][:])
                        nc.sync.dma_start(
                            out_d[dch, :, ds(tcb * 512, 512)], ot[:])

    nc.compile()
    return nc


def _get_program():
    global _PROGRAM
    if _PROGRAM is None:
        _PROGRAM = _build_program()
    return _PROGRAM


def _prep_inputs(x, Wq, bq, Wk, bk, Wv, bv, Wo, bo):
    """Build the 8 per-core input maps (host-side sharding, free)."""
    bf = ml_dtypes.bfloat16
    f8 = ml_dtypes.float8_e4m3
    x = np.asarray(x, dtype=np.float32)
    WqT = np.ascontiguousarray(np.asarray(Wq, np.float32).T)  # [D, D]
    WkT = np.ascontiguousarray(np.asarray(Wk, np.float32).T)
    WvT = np.ascontiguousarray(np.asarray(Wv, np.float32).T)
    WoT = np.ascontiguousarray(np.asarray(Wo, np.float32).T)  # [D, D] (f, d)

    def wlayout(WT, fsl):
        # [D, F] slice -> [P, DC, F]: w[p, dc, f] = WT[dc*128+p, f]
        w = np.asarray(WT[:, fsl], np.float32).reshape(DC, P, F)
        return np.ascontiguousarray(w.transpose(1, 0, 2)).astype(bf)

    def w8layout(WT, fsl):
        # [D, F] slice -> [P, DCP, 2, F] fp8, prescaled
        w = np.asarray(WT[:, fsl], np.float32).reshape(DCP, 2, P, F)
        w = np.ascontiguousarray(w.transpose(2, 0, 1, 3)) * W8SCALE
        return w.astype(f8)

    in_maps = []
    for c in range(NCORES):
        b, g = divmod(c, GROUPS)
        fsl = slice(g * F, (g + 1) * F)
        # x[b].T is [D, T]; bf16 tile layout [tcb, p, dc, t512]
        xTf = np.asarray(x[b].T, np.float32).reshape(DC, P, 4, 512)
        xT = np.ascontiguousarray(xTf.transpose(2, 1, 0, 3)).astype(bf)
        # fp8 pair layout [tcb, p, dcp, 2, t512]
        x8 = np.ascontiguousarray(
            xTf.reshape(DCP, 2, P, 4, 512).transpose(3, 2, 0, 1, 4)
        ).astype(f8)
        # wo: [F, D] slice -> [DC, P, HG, P]
        woc = np.asarray(WoT[fsl, :], np.float32).reshape(HG, P, DC, P)
        woc = np.ascontiguousarray(woc.transpose(2, 1, 0, 3)).astype(bf)
        m = {
            "xT": xT,
            "x8": x8,
            "wq": w8layout(WqT, fsl),
            "wk": w8layout(WkT, fsl),
            "wv": wlayout(WvT, fsl),
            "wo": woc,
            "bq": np.ascontiguousarray(
                np.asarray(bq, np.float32)[fsl].reshape(HG, P).T),
            "bk": np.ascontiguousarray(
                np.asarray(bk, np.float32)[fsl].reshape(HG, P).T),
        }
        in_maps.append(m)
    return in_maps


def _combine(results, bo_eff):
    out = np.empty((B, T, D), dtype=np.float32)
    for b in range(B):
        oT = (results[b * GROUPS]["out"].reshape(D, T).astype(np.float32)
              + results[b * GROUPS + 1]["out"].reshape(D, T).astype(np.float32))
        out[b] = oT.T + bo_eff[None, :]
    return out


def kernel(x, Wq, bq, Wk, bk, Wv, bv, Wo, bo):
    from concourse.bass_utils import run_bass_kernel_spmd

    nc = _get_program()
    in_maps = _prep_inputs(x, Wq, bq, Wk, bk, Wv, bv, Wo, bo)
    res = run_bass_kernel_spmd(nc, in_maps, list(range(NCORES))).results
    # v bias folded here: softmax rows sum to 1, so y_true = y_raw + bv
    # and out += bv @ Wo.T, a constant per output channel.
    bo_eff = (np.asarray(bo, np.float64)
              + np.asarray(Wo, np.float64) @ np.asarray(bv, np.float64))
    return _combine(res, bo_eff.astype(np.float32))


# revision 36
# speedup vs baseline: 1.0042x; 1.0042x over previous
"""Trainium2 Bass kernel: multi-head attention (B=4, T=2048, D=2048, H=16).

Sharding: 8 cores = 4 batches x 2 head-groups (tensor-parallel heads, data-
parallel batch). Each core handles one batch and 8 heads (f-slice of 1024
columns of the QKV projections / rows of the out-projection). Host sums the
two partial out-projection results per batch and adds the output bias.

Structure (v9):
  v-pass (bf16, dc-outer start so matmuls pace with DMA arrival), then
  q/k passes in fp8-e4m3 DoubleRow (256-contraction pair matmuls, weights
  host-prescaled by 64, descaled in the bias ACT; adds ~1.6e-2 rel err,
  inside the 2e-2 budget).  Attention units (head, q-half) are emitted
  half-major and software-pipelined (scores(u+1) before pv(u)) so the
  ScalarE exp stream never blocks the PE queue: S^T strips -> exp -> P^T;
  PV with ones-augmented V gives rowsums in col 129; normalize on DVE and
  DMA-xbar-transpose straight into yT (v bias folded into the host
  combine: softmax rows sum to 1, so out += Wo @ bv, a constant).  The
  out-projection for t-blocks 0/1 interleaves into the last units' PE
  slack (B is ScalarE-bound); t-blocks 2/3 run after.  DRAM layouts match
  SBUF tiles for large DMA packets; yT/pt pools reuse the w/x SBUF region.
"""

import sys

if "/opt/trn_rl_repo" not in sys.path:
    sys.path.insert(0, "/opt/trn_rl_repo")

import numpy as np
import ml_dtypes

D = 2048          # d_model
T = 2048          # sequence length
B = 4             # batch
H = 16            # total heads
DH = 128          # head dim
GROUPS = 2        # head groups (tensor-parallel factor per batch)
HG = H // GROUPS  # heads per core = 8
F = HG * DH       # per-core projection width = 1024
P = 128
DC = D // P       # 16 contraction chunks
DCP = DC // 2     # 8 fp8 DoubleRow pair-chunks
TC = T // P       # 16 t chunks
NCORES = 8
SCALE = float(1.0 / np.sqrt(DH))
W8SCALE = 64.0    # host prescale for fp8 weights (avoids e4m3 subnormals)


_PROGRAM = None


def _build_program():
    import concourse.bass as bass
    import concourse.tile as tile
    from concourse import bacc, mybir
    from concourse.bass import ts, ds
    from concourse.masks import make_identity

    bf16 = mybir.dt.bfloat16
    f32 = mybir.dt.float32
    f8 = mybir.dt.float8e4
    DR = mybir.MatmulPerfMode.DoubleRow

    nc = bacc.Bacc("TRN2", target_bir_lowering=False, debug=False,
                   num_devices=NCORES)

    # DRAM layouts match the SBUF tile layouts exactly (contiguous
    # per-partition runs -> large DMA packets).
    xT_d = nc.dram_tensor("xT", [4, P, DC, 512], bf16, kind="ExternalInput")
    x8_d = nc.dram_tensor("x8", [4, P, DCP, 2, 512], f8, kind="ExternalInput")
    wq_d = nc.dram_tensor("wq", [P, DCP, 2, F], f8, kind="ExternalInput")
    wk_d = nc.dram_tensor("wk", [P, DCP, 2, F], f8, kind="ExternalInput")
    wv_d = nc.dram_tensor("wv", [P, DC, F], bf16, kind="ExternalInput")
    wo_d = nc.dram_tensor("wo", [DC, P, HG, P], bf16, kind="ExternalInput")
    bq_d = nc.dram_tensor("bq", [P, HG], f32, kind="ExternalInput")
    bk_d = nc.dram_tensor("bk", [P, HG], f32, kind="ExternalInput")
    out_d = nc.dram_tensor("out", [DC, P, T], f32, kind="ExternalOutput")

    Exp = mybir.ActivationFunctionType.Exp
    Identity = mybir.ActivationFunctionType.Identity

    with tile.TileContext(nc) as tc:
        from contextlib import ExitStack
        with ExitStack() as ctx:
            # ---- persistent pools (allocated first, live whole kernel) ----
            const = ctx.enter_context(tc.tile_pool(name="const", bufs=1))
            qkt = ctx.enter_context(tc.tile_pool(name="qkt", bufs=1))
            vpool = ctx.enter_context(tc.tile_pool(name="vpool", bufs=1))

            zero_b = const.tile([P, 1], f32, tag="zerob")
            nc.vector.memset(zero_b[:], 0.0)
            bq_sb = const.tile([P, HG], f32, tag="bq")
            bk_sb = const.tile([P, HG], f32, tag="bk")

            qT = [qkt.tile([P, T], bf16, tag=f"qT{h}", name=f"qT{h}")
                  for h in range(HG)]
            kT = [qkt.tile([P, T], bf16, tag=f"kT{h}", name=f"kT{h}")
                  for h in range(HG)]
            v_sb = vpool.tile([P, TC, HG, DH + 1], bf16, tag="v")

            # force early allocation of persistent pools (first-use order)
            nc.vector.memset(qT[0][:, 0:1], 0.0)
            nc.vector.memset(v_sb[:, :, :, DH:DH + 1], 1.0)

            # ---------------- Phase A: projections ----------------
            with tc.tile_pool(name="wall", bufs=1) as wall, \
                 tc.tile_pool(name="x8p", bufs=2) as x8pool, \
                 tc.tile_pool(name="ps_a", bufs=8, space="PSUM") as ps_a:
                w_v = wall.tile([P, DC, F], bf16, tag="wv", name="w_v")
                w_q8 = wall.tile([P, DCP, 2, F], f8, tag="wq8", name="w_q8")

                # ---- v-pass (bf16, x streamed) ----
                # x8p allocated before vx so their regions don't overlap
                # (overlap would stall the x8 prefetch until v-pass ends).
                # wk8 is loaded later, into the region vx frees.
                x8_first = x8pool.tile([P, DCP, 2, 512], f8, tag="x8",
                                       name="x8_q0")
                with tc.tile_pool(name="vx", bufs=2) as vxpool:
                    xb0 = vxpool.tile([P, DC, 512], bf16, tag="xblk",
                                      name="xv0")
                    # startup: interleave wv and x chunks per-dc so the
                    # dc-outer matmuls pace with DMA arrival
                    for dc in range(DC):
                        nc.sync.dma_start(w_v[:, ds(dc, 1)],
                                          wv_d[:, ds(dc, 1)])
                        nc.sync.dma_start(xb0[:, ds(dc, 1)],
                                          xT_d[0, :, ds(dc, 1)])
                        if dc == 9:
                            xb1 = vxpool.tile([P, DC, 512], bf16, tag="xblk",
                                              name="xv1")
                            nc.sync.dma_start(xb1[:], xT_d[1])
                    # fp8 q weights + first x8 block: prefetch during v
                    nc.sync.dma_start(w_q8[:], wq_d[:])
                    nc.sync.dma_start(x8_first[:], x8_d[0])
                    nc.sync.dma_start(bq_sb[:], bq_d[:])
                    nc.sync.dma_start(bk_sb[:], bk_d[:])

                    for tcb in range(4):
                        if tcb == 0:
                            xblk = xb0
                        elif tcb == 1:
                            xblk = xb1
                        else:
                            xblk = vxpool.tile([P, DC, 512], bf16, tag="xblk",
                                               name=f"xv{tcb}")
                            nc.sync.dma_start(xblk[:], xT_d[tcb])
                        if tcb == 0:
                            # dc-outer so compute paces with DMA arrival
                            pls = [ps_a.tile([P, 512], f32, tag="ps512",
                                             name=f"v0l{t}") for t in range(4)]
                            prs = [ps_a.tile([P, 512], f32, tag="ps512",
                                             name=f"v0r{t}") for t in range(4)]
                            for dc in range(DC):
                                for tsub in range(4):
                                    lhs = xblk[:, dc, ds(tsub * P, P)]
                                    nc.tensor.matmul(
                                        pls[tsub][:], lhs, w_v[:, dc, 0:512],
                                        start=(dc == 0), stop=(dc == DC - 1))
                                    nc.tensor.matmul(
                                        prs[tsub][:], lhs, w_v[:, dc, 512:1024],
                                        start=(dc == 0), stop=(dc == DC - 1))
                            for tsub in range(4):
                                nc.vector.tensor_copy(
                                    v_sb[:, tsub, 0:4, 0:DH],
                                    pls[tsub][:].rearrange(
                                        "p (h d) -> p h d", d=DH))
                                nc.vector.tensor_copy(
                                    v_sb[:, tsub, 4:8, 0:DH],
                                    prs[tsub][:].rearrange(
                                        "p (h d) -> p h d", d=DH))
                        else:
                            for tsub in range(4):
                                tc_ = tcb * 4 + tsub
                                psl = ps_a.tile([P, 512], f32, tag="ps512",
                                                name=f"psl{tc_}")
                                psr = ps_a.tile([P, 512], f32, tag="ps512",
                                                name=f"psr{tc_}")
                                for dc in range(DC):
                                    lhs = xblk[:, dc, ds(tsub * P, P)]
                                    nc.tensor.matmul(
                                        psl[:], lhs, w_v[:, dc, 0:512],
                                        start=(dc == 0), stop=(dc == DC - 1))
                                    nc.tensor.matmul(
                                        psr[:], lhs, w_v[:, dc, 512:1024],
                                        start=(dc == 0), stop=(dc == DC - 1))
                                nc.vector.tensor_copy(
                                    v_sb[:, tc_, 0:4, 0:DH],
                                    psl[:].rearrange("p (h d) -> p h d", d=DH))
                                nc.vector.tensor_copy(
                                    v_sb[:, tc_, 4:8, 0:DH],
                                    psr[:].rearrange("p (h d) -> p h d", d=DH))

                # ---- q/k passes (fp8 DoubleRow, 256-contraction) ----
                # wk8 loads into the region vx freed, during the q-pass
                with tc.tile_pool(name="wk8p", bufs=1) as wk8pool:
                    w_k8 = wk8pool.tile([P, DCP, 2, F], f8, tag="wk8",
                                        name="w_k8")
                    nc.sync.dma_start(w_k8[:], wk_d[:])
                    for kind in ("q", "k"):
                        w8 = w_q8 if kind == "q" else w_k8
                        bias_sb = bq_sb if kind == "q" else bk_sb
                        dst = qT if kind == "q" else kT
                        for tcb in range(4):
                            if kind == "q" and tcb == 0:
                                x8 = x8_first
                            else:
                                x8 = x8pool.tile([P, DCP, 2, 512], f8,
                                                 tag="x8",
                                                 name=f"x8_{kind}{tcb}")
                                nc.sync.dma_start(x8[:], x8_d[tcb])
                            for h in range(HG):
                                ps = ps_a.tile([P, 512], f32, tag="ps512",
                                               name=f"ps_{kind}{tcb}{h}")
                                for j in range(DCP):
                                    nc.tensor.matmul(
                                        ps[:],
                                        w8[:, j, :, ds(h * DH, DH)],
                                        x8[:, j],
                                        start=(j == 0), stop=(j == DCP - 1),
                                        perf_mode=DR)
                                nc.scalar.activation(
                                    dst[h][:, ds(tcb * 512, 512)], ps[:],
                                    Identity, bias=bias_sb[:, ds(h, 1)],
                                    scale=1.0 / W8SCALE)

            # ---------------- Phase B: attention ----------------
            # yT + pt reuse the SBUF region freed by the A pools.
            ytp = ctx.enter_context(tc.tile_pool(name="ytp", bufs=1))
            yT = ytp.tile([P, HG, T], bf16, tag="yT")
            wop = ctx.enter_context(tc.tile_pool(name="wop", bufs=3))
            osb = ctx.enter_context(tc.tile_pool(name="osb", bufs=3))
            wocp = ctx.enter_context(tc.tile_pool(name="wocp", bufs=1))
            with tc.tile_pool(name="ptpool", bufs=2) as ptpool, \
                 tc.tile_pool(name="ystage", bufs=4) as ystage, \
                 tc.tile_pool(name="rspool", bufs=4) as rspool, \
                 tc.tile_pool(name="ps_st", bufs=2, space="PSUM") as ps_st, \
                 tc.tile_pool(name="ps_pv", bufs=2, space="PSUM") as ps_pv, \
                 tc.tile_pool(name="ps_o", bufs=2, space="PSUM") as ps_o:

                def emit_scores(h, half, fillers=()):
                    # S^T[k=128, q=1024] strips; exp -> P^T.  `fillers`
                    # (out-proj half-chunks) interleave between strips so
                    # they never block the strips feeding ScalarE for long.
                    q0 = half * (T // 2)
                    fillers = list(fillers)
                    pt = ptpool.tile([P, TC, T // 2], bf16, tag="pt",
                                     name=f"pt{h}_{half}")
                    for kc in range(TC):
                        st = ps_st.tile([P, T // 2], f32, tag="st",
                                        name=f"st{h}{half}{kc}")
                        for qc in range(2):
                            nc.tensor.matmul(
                                st[:, ds(qc * 512, 512)],
                                kT[h][:, ds(kc * P, P)],
                                qT[h][:, ds(q0 + qc * 512, 512)],
                                start=True, stop=True)
                        nc.scalar.activation(pt[:, kc], st[:], Exp,
                                             bias=zero_b[:, :],
                                             scale=SCALE)
                        if kc % 4 == 3 and fillers:
                            fillers.pop(0)()
                    for f in fillers:
                        f()
                    return pt

                def emit_pv(h, half, pt):
                    # PV: out[q=128, dh | rowsum]; normalize; DMA-xbar
                    # transpose straight into yT (bv folded into the host
                    # combine since softmax rows sum to 1)
                    q0 = half * (T // 2)
                    for qs in range(8):
                        pv = ps_pv.tile([P, DH + 1], f32, tag="pv",
                                        name=f"pv{h}{half}{qs}")
                        for kc in range(TC):
                            nc.tensor.matmul(
                                pv[:],
                                pt[:, kc, ds(qs * P, P)],
                                v_sb[:, kc, h],
                                start=(kc == 0), stop=(kc == TC - 1))
                        rs = rspool.tile([P, 1], f32, tag="rs",
                                         name=f"rs{h}{half}{qs}")
                        nc.vector.reciprocal(rs[:], pv[:, DH:DH + 1])
                        yst = ystage.tile([P, P], bf16, tag="yst",
                                          name=f"yst{h}{half}{qs}")
                        nc.vector.tensor_scalar_mul(yst[:], pv[:, 0:DH],
                                                    rs[:])
                        nc.sync.dma_start_transpose(
                            yT[:, h, ds(q0 + qs * P, P)], yst[:])

                def emit_outproj(dch, tcbs, wo_t):
                    # partial out-projection for t-blocks `tcbs`
                    for tcb in tcbs:
                        pso = ps_o.tile([P, 512], f32, tag="pso",
                                        name=f"pso{dch}_{tcb}")
                        for fc in range(HG):
                            nc.tensor.matmul(
                                pso[:],
                                wo_t[:, fc],
                                yT[:, fc, ds(tcb * 512, 512)],
                                start=(fc == 0), stop=(fc == HG - 1))
                        ot = osb.tile([P, 512], f32, tag="ot",
                                      name=f"ot{dch}_{tcb}")
                        nc.vector.tensor_copy(ot[:], pso[:])
                        nc.sync.dma_start(
                            out_d[dch, :, ds(tcb * 512, 512)], ot[:])

                # software pipeline, half-major: after the 8 half0 units,
                # yT[:, :, 0:1024] is complete, so the out-projection for
                # t-blocks 0-1 interleaves into the remaining units' PE
                # slack (B is ScalarE-bound).
                units = [(h, half) for half in range(2) for h in range(HG)]
                # C(t-blocks 0,1) interleave into units 9+ once half0 of
                # yT is complete; ~11 dch fit the ScalarE-bound B slack
                # (PE-B 229.6us + 38us of C ~= exp 268us), tapered so the
                # ScalarE lead buffer absorbs the PE overshoot.
                cplan = {i: 2 for i in range(9, 16)}
                prev = None
                next_dch = 0
                woc0 = None
                for i, (h, half) in enumerate(units):
                    fillers = []
                    for _ in range(cplan.get(i, 0)):
                        dch = next_dch
                        next_dch += 1
                        wo_t = wop.tile([P, HG, P], bf16, tag="wo",
                                        name=f"wo{dch}")

                        def f_a(dch=dch, wo_t=wo_t):
                            nc.sync.dma_start(wo_t[:], wo_d[dch])
                            emit_outproj(dch, (0,), wo_t)

                        def f_b(dch=dch, wo_t=wo_t):
                            emit_outproj(dch, (1,), wo_t)

                        fillers += [f_a, f_b]
                    pt = emit_scores(h, half, fillers)
                    if prev is not None:
                        emit_pv(prev[0], prev[1], prev[2])
                    prev = (h, half, pt)
                    if i == 14:
                        # dedicated prefetch of the C-tail's first wo so
                        # phase C isn't gated on a rotation-blocked DMA
                        woc0 = wocp.tile([P, HG, P], bf16, tag="woc0",
                                         name="woc0")
                        nc.sync.dma_start(woc0[:], wo_d[0])
                emit_pv(prev[0], prev[1], prev[2])
                for dch in range(next_dch, DC):
                    wo_t = wop.tile([P, HG, P], bf16, tag="wo",
                                    name=f"wo{dch}")
                    nc.sync.dma_start(wo_t[:], wo_d[dch])
                    emit_outproj(dch, (0, 1), wo_t)

            # ------------- Phase C: out-projection t-blocks 2,3 -------------
            with tc.tile_pool(name="ps_c", bufs=8, space="PSUM") as ps_c:
                for dch in range(DC):
                    if dch == 0:
                        wo_t = woc0
                    else:
                        wo_t = wop.tile([P, HG, P], bf16, tag="wo",
                                        name=f"wo_c{dch}")
                        nc.sync.dma_start(wo_t[:], wo_d[dch])
                    pso = [ps_c.tile([P, 512], f32, tag="psoc",
                                     name=f"psoc{dch}_{i}")
                           for i in range(2)]
                    for fc in range(HG):
                        for i, tcb in enumerate((2, 3)):
                            nc.tensor.matmul(
                                pso[i][:],
                                wo_t[:, fc],
                                yT[:, fc, ds(tcb * 512, 512)],
                                start=(fc == 0), stop=(fc == HG - 1))
                    for i, tcb in enumerate((2, 3)):
                        ot = osb.tile([P, 512], f32, tag="ot",
                                      name=f"otc{dch}_{tcb}")
                        nc.vector.tensor_copy(ot[:], pso[i][:])
                        nc.sync.dma_start(
                            out_d[dch, :, ds(tcb * 512, 512)], ot[:])

    nc.compile()
    return nc


def _get_program():
    global _PROGRAM
    if _PROGRAM is None:
        _PROGRAM = _build_program()
    return _PROGRAM


def _prep_inputs(x, Wq, bq, Wk, bk, Wv, bv, Wo, bo):
    """Build the 8 per-core input maps (host-side sharding, free)."""
    bf = ml_dtypes.bfloat16
    f8 = ml_dtypes.float8_e4m3
    x = np.asarray(x, dtype=np.float32)
    WqT = np.ascontiguousarray(np.asarray(Wq, np.float32).T)  # [D, D]
    WkT = np.ascontiguousarray(np.asarray(Wk, np.float32).T)
    WvT = np.ascontiguousarray(np.asarray(Wv, np.float32).T)
    WoT = np.ascontiguousarray(np.asarray(Wo, np.float32).T)  # [D, D] (f, d)

    def wlayout(WT, fsl):
        # [D, F] slice -> [P, DC, F]: w[p, dc, f] = WT[dc*128+p, f]
        w = np.asarray(WT[:, fsl], np.float32).reshape(DC, P, F)
        return np.ascontiguousarray(w.transpose(1, 0, 2)).astype(bf)

    def w8layout(WT, fsl):
        # [D, F] slice -> [P, DCP, 2, F] fp8, prescaled
        w = np.asarray(WT[:, fsl], np.float32).reshape(DCP, 2, P, F)
        w = np.ascontiguousarray(w.transpose(2, 0, 1, 3)) * W8SCALE
        return w.astype(f8)

    in_maps = []
    for c in range(NCORES):
        b, g = divmod(c, GROUPS)
        fsl = slice(g * F, (g + 1) * F)
        # x[b].T is [D, T]; bf16 tile layout [tcb, p, dc, t512]
        xTf = np.asarray(x[b].T, np.float32).reshape(DC, P, 4, 512)
        xT = np.ascontiguousarray(xTf.transpose(2, 1, 0, 3)).astype(bf)
        # fp8 pair layout [tcb, p, dcp, 2, t512]
        x8 = np.ascontiguousarray(
            xTf.reshape(DCP, 2, P, 4, 512).transpose(3, 2, 0, 1, 4)
        ).astype(f8)
        # wo: [F, D] slice -> [DC, P, HG, P]
        woc = np.asarray(WoT[fsl, :], np.float32).reshape(HG, P, DC, P)
        woc = np.ascontiguousarray(woc.transpose(2, 1, 0, 3)).astype(bf)
        m = {
            "xT": xT,
            "x8": x8,
            "wq": w8layout(WqT, fsl),
            "wk": w8layout(WkT, fsl),
            "wv": wlayout(WvT, fsl),
            "wo": woc,
            "bq": np.ascontiguousarray(
                np.asarray(bq, np.float32)[fsl].reshape(HG, P).T),
            "bk": np.ascontiguousarray(
                np.asarray(bk, np.float32)[fsl].reshape(HG, P).T),
        }
        in_maps.append(m)
    return in_maps


def _combine(results, bo_eff):
    out = np.empty((B, T, D), dtype=np.float32)
    for b in range(B):
        oT = (results[b * GROUPS]["out"].reshape(D, T).astype(np.float32)
              + results[b * GROUPS + 1]["out"].reshape(D, T).astype(np.float32))
        out[b] = oT.T + bo_eff[None, :]
    return out


def kernel(x, Wq, bq, Wk, bk, Wv, bv, Wo, bo):
    from concourse.bass_utils import run_bass_kernel_spmd

    nc = _get_program()
    in_maps = _prep_inputs(x, Wq, bq, Wk, bk, Wv, bv, Wo, bo)
    res = run_bass_kernel_spmd(nc, in_maps, list(range(NCORES))).results
    # v bias folded here: softmax rows sum to 1, so y_true = y_raw + bv
    # and out += bv @ Wo.T, a constant per output channel.
    bo_eff = (np.asarray(bo, np.float64)
              + np.asarray(Wo, np.float64) @ np.asarray(bv, np.float64))
    return _combine(res, bo_eff.astype(np.float32))
